# revision 40
# baseline (speedup 1.0000x reference)
"""AdaptiveQuantizedLinear on 8 TRN2 NeuronCores.

y = x @ W^T + bias, where W = ((W_q - zeros_g) * scales_g) * scale2 * mask.

Strategy (column-parallel / tensor-parallel over out_features):
 - Shard W-side tensors into 8 row-shards of OS=1376; replicate x.
 - Permuted contraction order: host reorders the in-feature axis as
   i(k, p) = (p//2)*64 + (p%2)*32 + k so that within every 128-row
   contraction tile the quant group depends only on the partition
   (gamma(p) = p//2). The per-group scale/zero tensors then broadcast to
   one pair of [128, OS] bf16 tiles built ONCE (instead of per k-tile),
   and the host-side reorder is a pure row permutation of x^T / W_q^T /
   mask^T (layout only; all arithmetic stays on device).
 - Per core: dequantize the W shard into a SBUF-resident bf16
   W^T [128 x 32 x 1376] (3 DVE tensor_tensor ops per k-tile, all in the
   bf16 2x mode via a u8->bf16 cast in the load DMA), then stream x^T
   tiles (f32->bf16 cast in DMA) through 32-long K-tile matmul chains
   accumulating in PSUM per <=512-wide output chunk; bias
   (partition-broadcast) is added during the PSUM->SBUF copy; f32
   results DMA out. The first 4 token tiles run k-outermost so the PE
   chases the dequant frontier instead of idling.
 - Host gathers the 8 [8192, 1376] f32 shards and reshapes.
"""
import numpy as np

import concourse.bass as bass
import concourse.mybir as mybir
from concourse import bacc, tile
from concourse.bass_utils import run_bass_kernel_spmd

B, S, I, O = 4, 2048, 4096, 11008
T = B * S                  # 8192 tokens
G = 64                     # quant group size
NG = I // G                # 64 groups
N_CORES = 8
OS = O // N_CORES          # 1376 out-features per core (free dim)
KT = I // 128              # 32 contraction tiles
TB = T // 256              # 32 token blocks (x DMA granularity)
OC = [(0, 512), (512, 512), (1024, 352)]  # output chunks (<=512 free dim)
AHEAD_TT = 4               # token tiles emitted k-outer during prologue
PPL = 128 * G // I         # partitions per quant group within a k-tile = 2

bf16 = mybir.dt.bfloat16
f32 = mybir.dt.float32
u8 = mybir.dt.uint8

# Set by test harnesses to capture HW profile; harmless by default.
TRACE = False
LAST_RESULT = None

_NC_CACHE = None


def _perm_idx():
    # host row 128*k + p holds original in-feature i(k, p)
    k = np.arange(KT)
    p = np.arange(128)
    i = (p[:, None] // PPL) * G + (p[:, None] % PPL) * (G // PPL) + k[None, :]
    return np.ascontiguousarray(i.T).reshape(-1)  # [(k, p)] -> i


def _build():
    nc = bacc.Bacc("TRN2", target_bir_lowering=False, debug=False,
                   num_devices=N_CORES)
    d_xT = nc.dram_tensor("xT", [I, T], f32, kind="ExternalInput")
    d_wm = nc.dram_tensor("wm", [I, 2, OS], u8, kind="ExternalInput")
    d_scT = nc.dram_tensor("scT", [NG, OS], f32, kind="ExternalInput")
    d_zeT = nc.dram_tensor("zeT", [NG, OS], f32, kind="ExternalInput")
    d_s2T = nc.dram_tensor("s2T", [1, OS], f32, kind="ExternalInput")
    d_b = nc.dram_tensor("bias", [OS], f32, kind="ExternalInput")
    d_y = nc.dram_tensor("y", [T, OS], f32, kind="ExternalOutput")

    with tile.TileContext(nc) as tc:
        with (
            tc.tile_pool(name="singles", bufs=1) as singles,
            tc.tile_pool(name="dram", bufs=1, space="DRAM") as drampool,
            tc.tile_pool(name="wpool", bufs=4) as wpool,
            tc.tile_pool(name="dqpool", bufs=2) as dqpool,
            tc.tile_pool(name="psum", bufs=1, space="PSUM") as psum,
            tc.tile_pool(name="xpool", bufs=3) as xpool,
            tc.tile_pool(name="opool", bufs=2) as opool,
        ):
            # resident dequantized W^T: [128 part (i within k-tile), KT, OS]
            WT = singles.tile([128, KT, OS], bf16)

            # scale2 is per-out-feature, so it commutes out of the matmul:
            # y = (x @ ((wq - z) * s * m)^T) * scale2 + bias. The group
            # scale/zero broadcasts therefore come straight from the input
            # tensors (f32 -> bf16 cast in the DMA), with no prep chain.
            s_bc = singles.tile([128, OS], bf16)
            nc.gpsimd.dma_start(
                out=s_bc,
                in_=bass.AP(tensor=d_scT, offset=0,
                            ap=[[OS, NG], [0, PPL], [1, OS]]))
            z_bc = singles.tile([128, OS], bf16)
            nc.gpsimd.dma_start(
                out=z_bc,
                in_=bass.AP(tensor=d_zeT, offset=0,
                            ap=[[OS, NG], [0, PPL], [1, OS]]))

            xT_r = d_xT.ap().rearrange("(k p) t -> p k t", p=128)

            # ---- prologue: dequantize W^T per k-tile, with the first two
            # x token-blocks loaded in k-chunks interleaved so neither
            # stream starves the DMA engines. The wm/x transfers are issued
            # ahead of the s_bc/z_bc broadcasts so they stream during the
            # scale-prep latency (SWDGE queue is FIFO per engine). ----
            xtbs = [xpool.tile([128, KT, 256], bf16, name=f"xtb_a{i_}",
                               tag="xtb") for i_ in range(2)]
            XCH = 8  # k-chunk of early x loads

            def load_x_chunk(tb, c):
                ks = slice(c * XCH, (c + 1) * XCH)
                nc.gpsimd.dma_start(
                    out=xtbs[tb][:, ks, :],
                    in_=xT_r[:, ks, tb * 256:(tb + 1) * 256])

            wm_tiles = {}

            def load_wm(k):
                # u8 -> bf16 cast in the DMA so every dequant op runs in
                # the DVE 2x mode (8-bit operands drop DVE to 1x)
                wm_t = wpool.tile([128, 2, OS], bf16, name=f"wm_{k}",
                                  tag="wm")
                nc.gpsimd.dma_start(
                    out=wm_t, in_=d_wm[k * 128:(k + 1) * 128, :, :])
                wm_tiles[k] = wm_t

            load_wm(0)
            load_x_chunk(0, 0)
            load_x_chunk(1, 0)
            WM_PF = 4
            for k in range(1, WM_PF):
                load_wm(k)

            s2_bc = singles.tile([128, OS], f32)
            nc.gpsimd.dma_start(
                out=s2_bc,
                in_=bass.AP(tensor=d_s2T, offset=0, ap=[[0, 128], [1, OS]]),
            )
            bias_bc = singles.tile([128, OS], f32)
            nc.gpsimd.dma_start(
                out=bias_bc,
                in_=bass.AP(tensor=d_b, offset=0, ap=[[0, 128], [1, OS]]),
            )

            for k in range(KT):
                wm_t = wm_tiles.pop(k)
                t1 = dqpool.tile([128, OS], bf16, tag="t1")
                nc.vector.tensor_tensor(out=t1, in0=wm_t[:, 0, :], in1=z_bc,
                                        op=mybir.AluOpType.subtract)
                nc.vector.tensor_tensor(out=t1, in0=t1, in1=s_bc,
                                        op=mybir.AluOpType.mult)
                nc.vector.tensor_tensor(out=WT[:, k, :], in0=t1,
                                        in1=wm_t[:, 1, :],
                                        op=mybir.AluOpType.mult)
                if k + WM_PF < KT:
                    load_wm(k + WM_PF)
                if k in (4, 12, 20):
                    c = k // 8 + 1
                    load_x_chunk(0, c)
                    load_x_chunk(1, c)

            # ---- phase A: first AHEAD_TT token tiles, k outermost so the
            # PE starts as soon as the first k-tiles are dequantized ----
            psA = {}
            for t in range(AHEAD_TT):
                for ci in (0, 1):
                    psA[(t, ci)] = psum.tile(
                        [128, OC[ci][1]], f32, name=f"psA_{t}_{ci}",
                        tag=f"ps{(2 * t + ci) % 8}")
            for k in range(KT):
                for t in range(AHEAD_TT):
                    xsl = xtbs[t // 2][:, k, (t % 2) * 128:(t % 2) * 128 + 128]
                    for ci in (0, 1):
                        o0, on = OC[ci]
                        nc.tensor.matmul(
                            psA[(t, ci)], lhsT=xsl, rhs=WT[:, k, o0:o0 + on],
                            start=(k == 0), stop=(k == KT - 1),
                        )

            def finish_chunk(ps, out_sb, ci):
                o0, on = OC[ci]
                nc.vector.tensor_tensor(
                    out=out_sb[:, o0:o0 + on], in0=ps,
                    in1=s2_bc[:, o0:o0 + on], op=mybir.AluOpType.mult)
                nc.vector.tensor_tensor(
                    out=out_sb[:, o0:o0 + on], in0=out_sb[:, o0:o0 + on],
                    in1=bias_bc[:, o0:o0 + on], op=mybir.AluOpType.add)

            nps = AHEAD_TT * 2
            # drain phase-A tiles: bias-add chunks 0/1, run chunk 2
            # (k innermost; WT is ready now), then store
            for t in range(AHEAD_TT):
                out_sb = opool.tile([128, OS], f32, name=f"outA_{t}",
                                    tag="out")
                for ci in (0, 1):
                    finish_chunk(psA[(t, ci)], out_sb, ci)
                o0, on = OC[2]
                ps = psum.tile([128, on], f32, tag=f"ps{nps % 8}")
                nps += 1
                xsl_t = xtbs[t // 2]
                for k in range(KT):
                    nc.tensor.matmul(
                        ps, lhsT=xsl_t[:, k, (t % 2) * 128:(t % 2) * 128 + 128],
                        rhs=WT[:, k, o0:o0 + on],
                        start=(k == 0), stop=(k == KT - 1))
                finish_chunk(ps, out_sb, 2)
                nc.sync.dma_start(
                    out=d_y[t * 128:(t + 1) * 128, :], in_=out_sb)

            # ---- phase B: remaining token tiles ----
            for tb in range(AHEAD_TT // 2, TB):
                xtb = xpool.tile([128, KT, 256], bf16, tag="xtb")
                nc.gpsimd.dma_start(
                    out=xtb, in_=xT_r[:, :, tb * 256:(tb + 1) * 256])
                for tloc in (0, 1):
                    tt = 2 * tb + tloc
                    out_sb = opool.tile([128, OS], f32, tag="out")
                    for ci, (o0, on) in enumerate(OC):
                        ps = psum.tile([128, on], f32, tag=f"ps{nps % 8}")
                        nps += 1
                        for k in range(KT):
                            nc.tensor.matmul(
                                ps,
                                lhsT=xtb[:, k, tloc * 128:tloc * 128 + 128],
                                rhs=WT[:, k, o0:o0 + on],
                                start=(k == 0), stop=(k == KT - 1))
                        finish_chunk(ps, out_sb, ci)
                    nc.sync.dma_start(
                        out=d_y[tt * 128:(tt + 1) * 128, :], in_=out_sb)

    nc.finalize()
    return nc


def _get_nc():
    global _NC_CACHE
    if _NC_CACHE is None:
        _NC_CACHE = _build()
    return _NC_CACHE


def kernel(x, scales, zeros, scale2, bias, W_q, mask):
    global LAST_RESULT
    idx = _perm_idx()
    x = np.asarray(x, dtype=np.float32).reshape(T, I)
    xT = np.ascontiguousarray(x.T)[idx]
    wq_u8 = np.asarray(W_q).astype(np.uint8)
    mask_u8 = np.asarray(mask).astype(np.uint8)
    scales = np.asarray(scales, dtype=np.float32)
    zeros = np.asarray(zeros, dtype=np.float32)
    scale2 = np.asarray(scale2, dtype=np.float32)
    bias = np.asarray(bias, dtype=np.float32)

    in_maps = []
    for c in range(N_CORES):
        r = slice(c * OS, (c + 1) * OS)
        wm = np.empty((I, 2, OS), np.uint8)
        wm[:, 0, :] = wq_u8[r].T[idx]
        wm[:, 1, :] = mask_u8[r].T[idx]
        in_maps.append({
            "xT": xT,
            "wm": wm,
            "scT": np.ascontiguousarray(scales[r].T),
            "zeT": np.ascontiguousarray(zeros[r].T),
            "s2T": np.ascontiguousarray(scale2[r].T),
            "bias": np.ascontiguousarray(bias[r]),
        })

    nc = _get_nc()
    res = run_bass_kernel_spmd(nc, in_maps, core_ids=list(range(N_CORES)),
                               trace=TRACE)
    LAST_RESULT = res
    y = np.concatenate([res.results[c]["y"] for c in range(N_CORES)], axis=1)
    return np.ascontiguousarray(y).reshape(B, S, O)


# revision 47
# speedup vs baseline: 1.1810x; 1.1810x over previous
"""AdaptiveQuantizedLinear on 8 TRN2 NeuronCores.

y = x @ W^T + bias, where W = ((W_q - zeros_g) * scales_g) * scale2 * mask.

Strategy (column-parallel / tensor-parallel over out_features):
 - Shard W-side tensors into 8 row-shards of OS=1376; replicate x.
 - Permuted contraction order: host reorders the in-feature axis as
   i(k, p) = (p//2)*64 + (p%2)*32 + k so that within every 128-row
   contraction tile the quant group depends only on the partition
   (gamma(p) = p//2). The per-group scale/zero tensors then broadcast to
   one pair of [128, OS] bf16 tiles built ONCE (instead of per k-tile),
   and the host-side reorder is a pure row permutation of x^T / W_q^T /
   mask^T (layout only; all arithmetic stays on device).
 - Per core: dequantize the W shard into a SBUF-resident bf16
   W^T [128 x 32 x 1376] (3 DVE tensor_tensor ops per k-tile, all in the
   bf16 2x mode via a u8->bf16 cast in the load DMA), then stream x^T
   tiles (f32->bf16 cast in DMA) through 32-long K-tile matmul chains
   accumulating in PSUM per <=512-wide output chunk; bias
   (partition-broadcast) is added during the PSUM->SBUF copy; f32
   results DMA out. The first 4 token tiles run k-outermost so the PE
   chases the dequant frontier instead of idling.
 - Host gathers the 8 [8192, 1376] f32 shards and reshapes.
"""
import numpy as np

import concourse.bass as bass
import concourse.mybir as mybir
from concourse import bacc, tile
from concourse.bass_utils import run_bass_kernel_spmd

B, S, I, O = 4, 2048, 4096, 11008
T = B * S                  # 8192 tokens
G = 64                     # quant group size
NG = I // G                # 64 groups
N_CORES = 8
OS = O // N_CORES          # 1376 out-features per core (free dim)
KT = I // 128              # 32 contraction tiles
TB = T // 256              # 32 token blocks (x DMA granularity)
OC = [(0, 512), (512, 512), (1024, 352)]  # output chunks (<=512 free dim)
AHEAD_TT = 4               # token tiles emitted k-outer during prologue
PPL = 128 * G // I         # partitions per quant group within a k-tile = 2

bf16 = mybir.dt.bfloat16
f32 = mybir.dt.float32
u8 = mybir.dt.uint8

# Set by test harnesses to capture HW profile; harmless by default.
TRACE = False
LAST_RESULT = None

_NC_CACHE = None


def _perm_idx():
    # host row 128*k + p holds original in-feature i(k, p)
    k = np.arange(KT)
    p = np.arange(128)
    i = (p[:, None] // PPL) * G + (p[:, None] % PPL) * (G // PPL) + k[None, :]
    return np.ascontiguousarray(i.T).reshape(-1)  # [(k, p)] -> i


def _build():
    nc = bacc.Bacc("TRN2", target_bir_lowering=False, debug=False,
                   num_devices=N_CORES)
    d_xT = nc.dram_tensor("xT", [I, T], f32, kind="ExternalInput")
    d_wm = nc.dram_tensor("wm", [I, 2, OS], u8, kind="ExternalInput")
    d_scT = nc.dram_tensor("scT", [NG, OS], f32, kind="ExternalInput")
    d_zeT = nc.dram_tensor("zeT", [NG, OS], f32, kind="ExternalInput")
    d_s2T = nc.dram_tensor("s2T", [1, OS], f32, kind="ExternalInput")
    d_b = nc.dram_tensor("bias", [OS], f32, kind="ExternalInput")
    d_y = nc.dram_tensor("y", [T, OS], f32, kind="ExternalOutput")

    with tile.TileContext(nc) as tc:
        with (
            tc.tile_pool(name="singles", bufs=1) as singles,
            tc.tile_pool(name="dram", bufs=1, space="DRAM") as drampool,
            tc.tile_pool(name="wpool", bufs=4) as wpool,
            tc.tile_pool(name="dqpool", bufs=2) as dqpool,
            tc.tile_pool(name="psum", bufs=1, space="PSUM") as psum,
            tc.tile_pool(name="xpool", bufs=3) as xpool,
            tc.tile_pool(name="opool", bufs=2) as opool,
        ):
            # resident dequantized W^T: [128 part (i within k-tile), KT, OS]
            WT = singles.tile([128, KT, OS], bf16)

            # scale2 is per-out-feature, so fold it into the group-scale
            # broadcast: W = (wq - z) * (s * s2) * m. The group scale/zero
            # broadcasts come straight from the input tensors (f32 -> bf16
            # cast in the DMA), with no prep chain or DRAM bounce.
            s_raw = singles.tile([128, OS], bf16)
            nc.gpsimd.dma_start(
                out=s_raw,
                in_=bass.AP(tensor=d_scT, offset=0,
                            ap=[[OS, NG], [0, PPL], [1, OS]]))
            z_bc = singles.tile([128, OS], bf16)
            nc.gpsimd.dma_start(
                out=z_bc,
                in_=bass.AP(tensor=d_zeT, offset=0,
                            ap=[[OS, NG], [0, PPL], [1, OS]]))
            s2_bc = singles.tile([128, OS], f32)
            nc.gpsimd.dma_start(
                out=s2_bc,
                in_=bass.AP(tensor=d_s2T, offset=0, ap=[[0, 128], [1, OS]]),
            )
            s_bc = singles.tile([128, OS], bf16)
            nc.vector.tensor_tensor(out=s_bc, in0=s_raw, in1=s2_bc,
                                    op=mybir.AluOpType.mult)

            xT_r = d_xT.ap().rearrange("(k p) t -> p k t", p=128)

            # ---- prologue: dequantize W^T per k-tile, with the first two
            # x token-blocks loaded in k-chunks interleaved so neither
            # stream starves the DMA engines. The wm/x transfers are issued
            # ahead of the s_bc/z_bc broadcasts so they stream during the
            # scale-prep latency (SWDGE queue is FIFO per engine). ----
            xtbs = [xpool.tile([128, KT, 256], bf16, name=f"xtb_a{i_}",
                               tag="xtb") for i_ in range(2)]
            XCH = 8  # k-chunk of early x loads

            def load_x_chunk(tb, c):
                ks = slice(c * XCH, (c + 1) * XCH)
                nc.gpsimd.dma_start(
                    out=xtbs[tb][:, ks, :],
                    in_=xT_r[:, ks, tb * 256:(tb + 1) * 256])

            wm_tiles = {}

            def load_wm(k):
                # u8 -> bf16 cast in the DMA so every dequant op runs in
                # the DVE 2x mode (8-bit operands drop DVE to 1x)
                wm_t = wpool.tile([128, 2, OS], bf16, name=f"wm_{k}",
                                  tag="wm")
                nc.gpsimd.dma_start(
                    out=wm_t, in_=d_wm[k * 128:(k + 1) * 128, :, :])
                wm_tiles[k] = wm_t

            load_wm(0)
            load_x_chunk(0, 0)
            load_x_chunk(1, 0)
            WM_PF = 4
            for k in range(1, WM_PF):
                load_wm(k)

            bias_bc = singles.tile([128, OS], f32)
            nc.gpsimd.dma_start(
                out=bias_bc,
                in_=bass.AP(tensor=d_b, offset=0, ap=[[0, 128], [1, OS]]),
            )

            for k in range(KT):
                wm_t = wm_tiles.pop(k)
                t1 = dqpool.tile([128, OS], bf16, tag="t1")
                nc.vector.tensor_tensor(out=t1, in0=wm_t[:, 0, :], in1=z_bc,
                                        op=mybir.AluOpType.subtract)
                nc.vector.tensor_tensor(out=t1, in0=t1, in1=s_bc,
                                        op=mybir.AluOpType.mult)
                nc.vector.tensor_tensor(out=WT[:, k, :], in0=t1,
                                        in1=wm_t[:, 1, :],
                                        op=mybir.AluOpType.mult)
                if k + WM_PF < KT:
                    load_wm(k + WM_PF)
                if k in (4, 12, 20):
                    c = k // 8 + 1
                    load_x_chunk(0, c)
                    load_x_chunk(1, c)

            # ---- phase A: first AHEAD_TT token tiles, k outermost so the
            # PE starts as soon as the first k-tiles are dequantized ----
            psA = {}
            for t in range(AHEAD_TT):
                for ci in (0, 1):
                    psA[(t, ci)] = psum.tile(
                        [128, OC[ci][1]], f32, name=f"psA_{t}_{ci}",
                        tag=f"ps{(2 * t + ci) % 8}")
            for k in range(KT):
                for t in range(AHEAD_TT):
                    xsl = xtbs[t // 2][:, k, (t % 2) * 128:(t % 2) * 128 + 128]
                    for ci in (0, 1):
                        o0, on = OC[ci]
                        nc.tensor.matmul(
                            psA[(t, ci)], lhsT=xsl, rhs=WT[:, k, o0:o0 + on],
                            start=(k == 0), stop=(k == KT - 1),
                        )

            def finish_chunk(ps, out_sb, ci):
                o0, on = OC[ci]
                nc.vector.tensor_tensor(
                    out=out_sb[:, o0:o0 + on], in0=ps,
                    in1=bias_bc[:, o0:o0 + on], op=mybir.AluOpType.add)

            nps = AHEAD_TT * 2
            # drain phase-A tiles: bias-add chunks 0/1, run chunk 2
            # (k innermost; WT is ready now), then store
            for t in range(AHEAD_TT):
                out_sb = opool.tile([128, OS], f32, name=f"outA_{t}",
                                    tag="out")
                for ci in (0, 1):
                    finish_chunk(psA[(t, ci)], out_sb, ci)
                o0, on = OC[2]
                ps = psum.tile([128, on], f32, tag=f"ps{nps % 8}")
                nps += 1
                xsl_t = xtbs[t // 2]
                for k in range(KT):
                    nc.tensor.matmul(
                        ps, lhsT=xsl_t[:, k, (t % 2) * 128:(t % 2) * 128 + 128],
                        rhs=WT[:, k, o0:o0 + on],
                        start=(k == 0), stop=(k == KT - 1))
                finish_chunk(ps, out_sb, 2)
                nc.sync.dma_start(
                    out=d_y[t * 128:(t + 1) * 128, :], in_=out_sb)

            # ---- phase B: remaining token tiles ----
            for tb in range(AHEAD_TT // 2, TB):
                xtb = xpool.tile([128, KT, 256], bf16, tag="xtb")
                nc.gpsimd.dma_start(
                    out=xtb, in_=xT_r[:, :, tb * 256:(tb + 1) * 256])
                for tloc in (0, 1):
                    tt = 2 * tb + tloc
                    out_sb = opool.tile([128, OS], f32, tag="out")
                    for ci, (o0, on) in enumerate(OC):
                        ps = psum.tile([128, on], f32, tag=f"ps{nps % 8}")
                        nps += 1
                        for k in range(KT):
                            nc.tensor.matmul(
                                ps,
                                lhsT=xtb[:, k, tloc * 128:tloc * 128 + 128],
                                rhs=WT[:, k, o0:o0 + on],
                                start=(k == 0), stop=(k == KT - 1))
                        finish_chunk(ps, out_sb, ci)
                    nc.sync.dma_start(
                        out=d_y[tt * 128:(tt + 1) * 128, :], in_=out_sb)

    nc.finalize()
    return nc


def _get_nc():
    global _NC_CACHE
    if _NC_CACHE is None:
        _NC_CACHE = _build()
    return _NC_CACHE


def kernel(x, scales, zeros, scale2, bias, W_q, mask):
    global LAST_RESULT
    idx = _perm_idx()
    x = np.asarray(x, dtype=np.float32).reshape(T, I)
    xT = np.ascontiguousarray(x.T)[idx]
    wq_u8 = np.asarray(W_q).astype(np.uint8)
    mask_u8 = np.asarray(mask).astype(np.uint8)
    scales = np.asarray(scales, dtype=np.float32)
    zeros = np.asarray(zeros, dtype=np.float32)
    scale2 = np.asarray(scale2, dtype=np.float32)
    bias = np.asarray(bias, dtype=np.float32)

    in_maps = []
    for c in range(N_CORES):
        r = slice(c * OS, (c + 1) * OS)
        wm = np.empty((I, 2, OS), np.uint8)
        wm[:, 0, :] = wq_u8[r].T[idx]
        wm[:, 1, :] = mask_u8[r].T[idx]
        in_maps.append({
            "xT": xT,
            "wm": wm,
            "scT": np.ascontiguousarray(scales[r].T),
            "zeT": np.ascontiguousarray(zeros[r].T),
            "s2T": np.ascontiguousarray(scale2[r].T),
            "bias": np.ascontiguousarray(bias[r]),
        })

    nc = _get_nc()
    res = run_bass_kernel_spmd(nc, in_maps, core_ids=list(range(N_CORES)),
                               trace=TRACE)
    LAST_RESULT = res
    y = np.concatenate([res.results[c]["y"] for c in range(N_CORES)], axis=1)
    return np.ascontiguousarray(y).reshape(B, S, O)
